# revision 28
# baseline (speedup 1.0000x reference)
"""LorentzNet Trainium2 Bass kernel.

Strategy (data-parallel over events, 8 cores x 4 events):
  * Exploits the fixed complete intra-event edge structure (P=128 particles,
    all ordered pairs i!=j) to replace gather/scatter with dense per-event
    compute.
  * Pair space (128x128 = 16384 pairs per event, i-major) is processed in
    4 chunks of 8 blocks; each block = 512 pairs (4 i x 128 j).
  * Edge MLP runs feature-major ([72, pairs] tiles) on the PE with float32r
    matmuls (1 cycle/col at N=512).  Key tricks:
      - h_j stream via 0-stride tiled rhs AP on hT
      - A_i = (h @ We1a) broadcast via identity-tile rhs with 0-stride AP
      - norms/prods via Minkowski Gram matrix + psi, flattened to pair rows
      - w-row rides the Wx1 pass as output row 96 (M=97)
      - phi_x produced directly as PHI^T columns via small M=128/N=1 matmuls
      - segment-sum over j via strided DVE reduce; w broadcast via gpsimd
"""

import numpy as np

import concourse.bass as bass
import concourse.bacc as bacc
import concourse.mybir as mybir
from concourse.bass import AP
from concourse.tile import TileContext

f32 = mybir.dt.float32
f32r = mybir.dt.float32r
bf16 = mybir.dt.bfloat16
ACTF = mybir.ActivationFunctionType
ALU = mybir.AluOpType
AX = mybir.AxisListType

HID = 72
P = 128
L = 6
C_W = 0.005
B = 32
N_CORES = 8
EV = B // N_CORES          # events per core
F = 512                    # pairs per block (4 i x 128 j)
BLK_I = 4                  # i's per block
CHUNK_BLKS = 8             # blocks per chunk
NCHUNK = (P * P) // (F * CHUNK_BLKS)   # 4
CPAIR = F * CHUNK_BLKS     # 4096 pairs per chunk


def r(ap):
    return ap.bitcast(f32r)




def const_layout(n_ev, n_layers):
    """Ordered specs: name -> (dtype 'r'|'f', partitions, width)."""
    from collections import OrderedDict
    s = OrderedDict()
    s["maskdiv"] = ("r", P, n_ev)
    s["idm"] = ("r", P, P)
    s["phimaskT"] = ("f", P, P)
    s["etac"] = ("r", 4, 1)
    s["etac2"] = ("f", 4, 1)
    s["onesr"] = ("r", 1, P)
    s["embW"] = ("r", 4, HID)
    s["embb"] = ("f", HID, 1)
    s["decW1"] = ("r", HID, HID)
    s["db1"] = ("f", HID, 1)
    s["decW2"] = ("r", HID, 2)
    s["db2"] = ("f", 2, 1)
    for l in range(n_layers):
        s[f"l{l}_We1b"] = ("r", HID, HID)
        s[f"l{l}_uvW"] = ("r", 2, HID)
        s[f"l{l}_We1a"] = ("r", HID, HID)
        s[f"l{l}_be1"] = ("f", HID, 1)
        s[f"l{l}_We2"] = ("r", HID, HID)
        s[f"l{l}_be2"] = ("f", HID, 1)
        s[f"l{l}_WxM"] = ("r", HID, 97)
        s[f"l{l}_bx1"] = ("f", HID, 1)
        s[f"l{l}_bm97"] = ("f", 97, 1)
        s[f"l{l}_Wx2"] = ("r", HID, 1)
        s[f"l{l}_Wh1a"] = ("r", HID, HID)
        s[f"l{l}_Wh1b"] = ("r", HID, HID)
        s[f"l{l}_bh1"] = ("f", HID, 1)
        s[f"l{l}_Wh2"] = ("r", HID, HID)
        s[f"l{l}_bh2"] = ("f", HID, 1)
    return s


def const_layout_h(n_layers):
    """bf16 const pack: name -> (partitions, width)."""
    from collections import OrderedDict
    s = OrderedDict()
    s["wmrow"] = (1, P * P)
    for l in range(n_layers):
        s[f"l{l}_uvW"] = (2, HID)
        s[f"l{l}_WxM"] = (HID, 97)
        s[f"l{l}_Wh1b"] = (HID, HID)
    return s


def _offsets(specs):
    offs, off = [], 0
    for _, (_, _, w) in specs.items():
        offs.append(off)
        off += w
    return offs


def build_program(n_ev=EV, n_layers=L):
    nc = bacc.Bacc()

    # ---------------- DRAM I/O ----------------
    din = {}

    specs = const_layout(n_ev, n_layers)
    cpack_w = sum(w for _, _, w in specs.values())
    din["cpack"] = nc.dram_tensor("cpack", [P, cpack_w], f32r, kind="ExternalInput")
    hspecs = const_layout_h(n_layers)
    hpack_w = sum(w for _, w in hspecs.values())
    din["cpackh"] = nc.dram_tensor("cpackh", [P, hpack_w], bf16, kind="ExternalInput")
    din["momT"] = nc.dram_tensor("momT", [4, n_ev * P], f32r, kind="ExternalInput")
    din["scalT"] = nc.dram_tensor("scalT", [4, n_ev * P], f32r, kind="ExternalInput")
    out_d = nc.dram_tensor("out", [n_ev, 2], f32, kind="ExternalOutput")

    from contextlib import ExitStack
    with TileContext(nc) as tc, ExitStack() as stack:
        consts = stack.enter_context(tc.tile_pool(name="consts", bufs=1))
        state = stack.enter_context(tc.tile_pool(name="state", bufs=1))
        evp = stack.enter_context(tc.tile_pool(name="evp", bufs=2))
        chp = stack.enter_context(tc.tile_pool(name="chp", bufs=2))
        blk = stack.enter_context(tc.tile_pool(name="blk", bufs=4))
        psb = stack.enter_context(tc.tile_pool(name="psb", bufs=1, space="PSUM"))

        # ------- load packed constants with ONE DMA -------
        cpk = consts.tile([P, cpack_w], f32r, tag="cpack", name="cpack")
        nc.sync.dma_start(out=cpk, in_=din["cpack"][:, :])
        C = {}
        for off, (name, (dt_, p_, w_)) in zip(_offsets(specs), specs.items()):
            ap = cpk[0:p_, off:off + w_]
            C[name] = ap if dt_ == "r" else ap.bitcast(f32)
        cph = consts.tile([P, hpack_w], bf16, tag="cpackh", name="cpackh")
        nc.sync.dma_start(out=cph, in_=din["cpackh"][:, :])
        H = {}
        hoff = 0
        for name, (p_, w_) in hspecs.items():
            H[name] = cph[0:p_, hoff:hoff + w_]
            hoff += w_

        idm = C["idm"]

        # ------- persistent per-event state -------
        hT = [state.tile([HID, P], f32r, tag=f"hT{e}", name=f"hT{e}") for e in range(n_ev)]
        X = [state.tile([P, 4], f32r, tag=f"X{e}", name=f"X{e}") for e in range(n_ev)]

        # PSUM tags: pre(2) m(2) t2w(2) phit(1) misc(1) = 8 banks
        def ps_pre():
            return psb.tile([HID, F], f32, tag="pre", bufs=2, name="ps_pre")

        def ps_m():
            return psb.tile([HID, F], f32, tag="m", bufs=2, name="ps_m")

        def ps_t2w():
            return psb.tile([97, F], f32, tag="t2w", bufs=2, name="ps_t2w")

        def ps_phit():
            return psb.tile([P, P], f32, tag="phit", bufs=1, name="ps_phit")

        def ps_misc(shape):
            return psb.tile(shape, f32, tag="misc", bufs=1, name="ps_misc")

        # ---------------- embedding + x init ----------------
        for e in range(n_ev):
            xT = state.tile([4, P], f32r, tag=f"xT{e}")
            nc.sync.dma_start(out=xT, in_=din["momT"][:, e * P:(e + 1) * P])
            pm = ps_misc([P, 4])
            nc.tensor.transpose(pm[:], xT[:].bitcast(f32), idm[0:4, 0:4].bitcast(f32))
            nc.vector.tensor_copy(X[e][:], pm[:])

            sc = state.tile([4, P], f32r, tag=f"sc{e}")
            nc.sync.dma_start(out=sc, in_=din["scalT"][:, e * P:(e + 1) * P])
            ph = ps_misc([HID, P])
            nc.tensor.matmul(ph[:], r(C["embW"][:]), r(sc[:]), start=True, stop=True)
            nc.scalar.activation(hT[e][:], ph[:], ACTF.Identity, bias=C["embb"][:, 0:1])

        # ---------------- per (layer, event) phases ----------------
        def phase_pre(l, e):
            """Geometry: xT, Gram matrices, psi, npF; A_rows."""
            W = {k: C[f"l{l}_{k}"] for k in
                 ("We1b", "uvW", "We1a", "be1", "We2", "be2", "WxM", "bx1",
                  "bm97", "Wx2", "Wh1a", "Wh1b", "bh1", "Wh2", "bh2")}
            t = {}
            # xT = X^T
            t["xT"] = evp.tile([4, P], f32r, tag="exT", name="exT")
            pm = ps_misc([4, P])
            nc.tensor.transpose(pm[:], X[e][:].bitcast(f32), idm[:, :].bitcast(f32))
            nc.vector.tensor_copy(t["xT"][:], pm[:])
            xT = t["xT"]
            # squared / eta-scaled rows
            sqT = evp.tile([4, P], f32r, tag="sqT")
            nc.vector.tensor_mul(sqT[:], xT[:], xT[:])
            exT = evp.tile([4, P], f32r, tag="exT2")
            nc.vector.tensor_scalar(out=exT[:], in0=xT[:].bitcast(f32),
                                    scalar1=C["etac"][:, 0:1].bitcast(f32),
                                    scalar2=None, op0=ALU.mult)
            e2xT = evp.tile([4, P], f32r, tag="e2xT")
            nc.vector.tensor_scalar(out=e2xT[:], in0=xT[:].bitcast(f32),
                                    scalar1=C["etac2"][:, 0:1],
                                    scalar2=None, op0=ALU.mult)
            # n_row = eta . sq  -> [1, P]
            pn = ps_misc([1, P])
            nc.tensor.matmul(pn[:], r(C["etac"][:]), r(sqT[:]), start=True, stop=True)
            n_row = evp.tile([1, P], f32r, tag="n_row")
            nc.scalar.copy(n_row[:], pn[:])
            # n_col = transpose(n_row)
            pc = ps_misc([P, 1])
            nc.tensor.transpose(pc[:], n_row[:].bitcast(f32), idm[0:1, 0:1].bitcast(f32))
            n_col = evp.tile([P, 1], f32, tag="n_col")
            nc.vector.tensor_copy(n_col[:], pc[:])

            # prods: G = eta-gram; psi
            pg = ps_misc([P, P])
            nc.tensor.matmul(pg[:], r(exT[:]), r(xT[:]), start=True, stop=True)
            ga = evp.tile([P, P], f32, tag="ga")
            nc.scalar.activation(ga[:], pg[:], ACTF.Abs)
            gl = evp.tile([P, P], f32, tag="gl")
            nc.scalar.activation(gl[:], ga[:], ACTF.Ln, bias=1.0)
            gs = evp.tile([P, P], f32, tag="gs")
            nc.scalar.activation(gs[:], pg[:], ACTF.Sign)
            gpsi = evp.tile([P, P], bf16, tag="gpsi")
            nc.vector.tensor_mul(gpsi[:], gs[:], gl[:])

            # norms2 = n_i + n_j - 2G ; psi
            pq = ps_misc([P, P])
            nc.tensor.matmul(pq[:], r(e2xT[:]), r(xT[:]), start=True, stop=False)
            nc.tensor.matmul(pq[:], r(C["onesr"][:]), r(n_row[:]), start=False, stop=True)
            nz = evp.tile([P, P], f32, tag="nz")
            nc.scalar.activation(nz[:], pq[:], ACTF.Identity, bias=n_col[:, 0:1])
            na = evp.tile([P, P], f32, tag="na")
            nc.scalar.activation(na[:], nz[:], ACTF.Abs)
            nl = evp.tile([P, P], f32, tag="nl")
            nc.scalar.activation(nl[:], na[:], ACTF.Ln, bias=1.0)
            ns = evp.tile([P, P], f32, tag="ns")
            nc.scalar.activation(ns[:], nz[:], ACTF.Sign)
            npsi = evp.tile([P, P], bf16, tag="npsi")
            nc.vector.tensor_mul(npsi[:], ns[:], nl[:])

            npF = evp.tile([2, P * P], bf16, tag="npF", bufs=1, name="npF")
            nc.gpsimd.dma_start(out=npF[0:1, :], in_=npsi[:, :])
            nc.gpsimd.dma_start(out=npF[1:2, :], in_=gpsi[:, :])

            # A_rows = h @ We1a  [P, HID]
            pa = ps_misc([P, HID])
            nc.tensor.matmul(pa[:], hT[e][:].bitcast(f32), W["We1a"][:].bitcast(f32), start=True, stop=True)
            A_rows = evp.tile([P, HID], f32r, tag="A_rows")
            nc.vector.tensor_copy(A_rows[:], pa[:])

            t.update(npF=npF, A_rows=A_rows, W=W)
            return t

        def phase_A_chunk(l, e, c, ctx, interleave=None):
            """Edge MLP for one chunk (8 blocks of 512 pairs).
            interleave(kb) emits the previous chunk's C-block after each block."""
            W = ctx["W"]
            A_rows = ctx["A_rows"]
            npF = ctx["npF"]
            m_ch = chp.tile([HID, CPAIR], f32r, tag="m_ch")
            wsC = chp.tile([97, CPAIR], bf16, tag="wsC")
            phit_ps = ctx.get("phit_ps")
            if phit_ps is None and l < n_layers - 1:
                phit_ps = ps_phit()
                ctx["phit_ps"] = phit_ps
            h = hT[e]
            rhs_h = AP(h.tensor, h.offset, [h.ap[0], [0, BLK_I], [1, P]])
            for kb in range(CHUNK_BLKS):
                k = c * CHUNK_BLKS + kb
                i0 = k * BLK_I
                cc0 = kb * F
                pre = ps_pre()
                nc.tensor.matmul(pre[:], r(W["We1b"][:]), r(rhs_h), start=True, stop=False)
                rhsA = AP(idm.tensor, idm.offset + i0,
                          [idm.ap[0], [1, BLK_I], [0, P]])
                nc.tensor.matmul(pre[:], r(A_rows[:]), r(rhsA), start=False, stop=False)
                k_g = c * CHUNK_BLKS + kb
                nc.tensor.matmul(pre[:], H[f"l{l}_uvW"][:], npF[:, k_g * F:k_g * F + F],
                                 start=False, stop=True)
                t1 = blk.tile([HID, F], f32r, tag="t1")
                nc.scalar.activation(t1[:], pre[:], ACTF.Relu, bias=W["be1"][:, 0:1])
                pm = ps_m()
                nc.tensor.matmul(pm[:], r(W["We2"][:]), r(t1[:]), start=True, stop=True)
                nc.vector.tensor_scalar(out=m_ch[:, cc0:cc0 + F], in0=pm[:],
                                        scalar1=W["be2"][:, 0:1], scalar2=0.0,
                                        op0=ALU.add, op1=ALU.max)
                pt = ps_t2w()
                nc.tensor.matmul(pt[:], r(W["WxM"][:]), r(m_ch[:, cc0:cc0 + F]),
                                 start=True, stop=True)
                nc.scalar.activation(wsC[96:97, cc0:cc0 + F], pt[96:97, :],
                                     ACTF.Sigmoid, bias=W["bm97"][96:97, 0:1])
                if l < n_layers - 1:
                    t2 = blk.tile([HID, F], f32r, tag="t2")
                    nc.scalar.activation(t2[:], pt[0:72, :], ACTF.Relu,
                                         bias=W["bx1"][:, 0:1])
                    for il in range(BLK_I):
                        nc.tensor.matmul(phit_ps[:, i0 + il:i0 + il + 1],
                                         t2[:, il * P:(il + 1) * P].bitcast(f32),
                                         W["Wx2"][:].bitcast(f32),
                                         start=True, stop=True)
                if interleave is not None:
                    interleave(kb)
            wrow0 = chp.tile([1, CPAIR], bf16, tag="wrow0", name="wrow0")
            nc.sync.dma_start(out=wrow0[0:1, :], in_=wsC[96:97, :])
            nc.vector.tensor_mul(wrow0[0:1, :], wrow0[0:1, :],
                                 H["wmrow"][0:1, c * CPAIR:(c + 1) * CPAIR])
            ctx[f"m_ch{c}"] = m_ch
            ctx[f"wrow0{c}"] = wrow0

        def phase_C_block(l, e, c, kb, ctx):
            m_ch = ctx[f"m_ch{c}"]
            wrow0 = ctx[f"wrow0{c}"]
            wmT = ctx["wmT"]
            k = c * CHUNK_BLKS + kb
            i0 = k * BLK_I
            cc0 = kb * F
            wb = blk.tile([HID, F], bf16, tag="wb", name="wb")
            nc.gpsimd.partition_broadcast(wb[:], wrow0[0:1, cc0:cc0 + F])
            wp = blk.tile([HID, F], f32, tag="wp", name="wp")
            nc.vector.tensor_mul(wp[:], m_ch[:, cc0:cc0 + F].bitcast(f32), wb[:])
            with nc.allow_low_precision(reason="f32r wm accumulation"):
                nc.vector.reduce_sum(wmT[:, i0:i0 + BLK_I],
                                     wp[:].rearrange("p (g j) -> p g j", g=BLK_I),
                                     axis=AX.X)

        def phase_C_chunk(l, e, c, ctx):
            for kb in range(CHUNK_BLKS):
                phase_C_block(l, e, c, kb, ctx)

        def phase_Bev(l, e, ctx):
            """x update from PHI^T psum."""
            if l >= n_layers - 1:
                return
            phit_ps = ctx["phit_ps"]
            phiT = evp.tile([P, P], f32r, tag="phiT")
            nc.vector.tensor_mul(phiT[:], phit_ps[:], C["phimaskT"][:])
            pn = ps_misc([P, 4])
            nc.tensor.matmul(pn[:], phiT[:].bitcast(f32), X[e][:].bitcast(f32), start=True, stop=True)
            tmp = evp.tile([P, 4], f32, tag="xtmp")
            nc.vector.tensor_scalar(out=tmp[:], in0=pn[:], scalar1=C_W / (P - 1.0),
                                    scalar2=None, op0=ALU.mult)
            nc.vector.tensor_add(X[e][:], X[e][:].bitcast(f32), tmp[:])

        def phase_D(l, e, ctx):
            """h update."""
            W = ctx["W"]
            wmT = ctx["wmT"]
            ph = ps_misc([HID, P])
            nc.tensor.matmul(ph[:], r(W["Wh1a"][:]), r(hT[e][:]), start=True, stop=False)
            nc.tensor.matmul(ph[:], r(W["Wh1b"][:]), r(wmT[:]), start=False, stop=True)
            u1 = evp.tile([HID, P], f32r, tag="u1")
            nc.scalar.activation(u1[:], ph[:], ACTF.Relu, bias=W["bh1"][:, 0:1])
            ph2 = ps_misc([HID, P])
            nc.tensor.matmul(ph2[:], r(W["Wh2"][:]), r(u1[:]), start=True, stop=True)
            tmph = evp.tile([HID, P], f32, tag="tmph")
            nc.vector.tensor_scalar(out=tmph[:], in0=ph2[:], scalar1=W["bh2"][:, 0:1],
                                    scalar2=None, op0=ALU.add)
            nc.vector.tensor_add(hT[e][:], hT[e][:].bitcast(f32), tmph[:])

        # ---------------- emission with software pipelining ----------------
        pending = []   # deferred (Bev, D) emitters, tagged with event

        def flush_pending(ev=None):
            keep = []
            while pending:
                pe, fn = pending.pop(0)
                if ev is None or pe == ev:
                    fn()
                else:
                    keep.append((pe, fn))
            pending.extend(keep)

        for l in range(n_layers):
            for e in range(n_ev):
                # event e's deferred updates must precede its next-layer reads
                flush_pending(ev=e)
                ctx = phase_pre(l, e)
                ctx["wmT"] = evp.tile([HID, P], f32r, tag="wmT", name="wmT")
                phase_A_chunk(l, e, 0, ctx)
                # emit deferred work of previous event-layer here (overlaps)
                flush_pending()
                for c in range(1, NCHUNK):
                    phase_A_chunk(l, e, c, ctx,
                                  interleave=lambda kb, c=c: phase_C_block(
                                      l, e, c - 1, kb, ctx))
                phase_C_chunk(l, e, NCHUNK - 1, ctx)

                def tail(l=l, e=e, ctx=ctx):
                    phase_Bev(l, e, ctx)
                    phase_D(l, e, ctx)
                pending.append((e, tail))
        flush_pending()

        # ---------------- pooling + decoder ----------------
        pz = psb.tile([HID, n_ev], f32, tag="phit", bufs=1, name="pz")
        for e in range(n_ev):
            pr = ps_misc([P, HID])
            nc.tensor.transpose(pr[:], hT[e][:].bitcast(f32), idm[0:HID, 0:HID].bitcast(f32))
            hrow = evp.tile([P, HID], f32r, tag="hrow")
            nc.vector.tensor_copy(hrow[:], pr[:])
            nc.tensor.matmul(pz[:, e:e + 1], hrow[:].bitcast(f32),
                             C["maskdiv"][:, e:e + 1].bitcast(f32),
                             start=True, stop=True)
        Z = evp.tile([HID, n_ev], f32r, tag="Z")
        nc.scalar.copy(Z[:], pz[:])
        pd1 = ps_misc([HID, n_ev])
        nc.tensor.matmul(pd1[:], C["decW1"][:].bitcast(f32), Z[:].bitcast(f32), start=True, stop=True)
        D1 = evp.tile([HID, n_ev], f32r, tag="D1")
        nc.scalar.activation(D1[:], pd1[:], ACTF.Relu, bias=C["db1"][:, 0:1])
        pd2 = ps_misc([2, n_ev])
        nc.tensor.matmul(pd2[:], C["decW2"][:].bitcast(f32), D1[:].bitcast(f32), start=True, stop=True)
        osb = evp.tile([2, n_ev], f32, tag="osb")
        nc.scalar.activation(osb[:], pd2[:], ACTF.Identity, bias=C["db2"][:, 0:1])
        nc.sync.dma_start(out=out_d.rearrange("e k -> k e"), in_=osb[:])

    nc.finalize()
    return nc


# ====================== host-side packing ======================

def pack_inputs(mom4, mask, scalars, params, core, n_ev=EV, n_layers=L):
    """Build the per-core input map (numpy float32 arrays)."""
    e0 = core * n_ev
    sl = slice(e0, e0 + n_ev)
    d = {}
    d["momT"] = np.ascontiguousarray(
        mom4[sl].transpose(2, 0, 1).reshape(4, n_ev * P)).astype(np.float32)
    d["scalT"] = np.ascontiguousarray(
        scalars[sl].transpose(2, 0, 1).reshape(4, n_ev * P)).astype(np.float32)
    flags = (mask[sl, :, 0] != 0).astype(np.float32) / P      # [n_ev, P]
    d["maskdiv"] = np.ascontiguousarray(flags.T)              # [P, n_ev]
    d["idm"] = np.eye(P, dtype=np.float32)
    # wmaskC: [8, NCHUNK*F]; chunk c block kb covers i = 4*(8c+kb)+il, col (il, j)
    wmr = np.ones((1, P * P), dtype=np.float32)
    for i in range(P):
        wmr[0, i * P + i] = 0.0
    d["wmrow"] = wmr
    d["phimaskT"] = (1.0 - np.eye(P)).astype(np.float32)
    d["etac"] = np.array([[1.0], [-1.0], [-1.0], [-1.0]], dtype=np.float32)
    d["etac2"] = np.array([[-2.0], [2.0], [2.0], [2.0]], dtype=np.float32)
    d["onesr"] = np.ones((1, P), dtype=np.float32)
    d["embW"] = np.asarray(params["emb_W"], dtype=np.float32)
    d["embb"] = np.asarray(params["emb_b"], dtype=np.float32).reshape(HID, 1)
    d["decW1"] = np.asarray(params["dec_W1"], dtype=np.float32)
    d["db1"] = np.asarray(params["dec_b1"], dtype=np.float32).reshape(HID, 1)
    d["decW2"] = np.asarray(params["dec_W2"], dtype=np.float32)
    d["db2"] = np.asarray(params["dec_b2"], dtype=np.float32).reshape(2, 1)
    for l in range(n_layers):
        p = params["lgeb"][l]
        We1 = np.asarray(p["We1"], dtype=np.float32)          # [146, 72]
        d[f"l{l}_We1b"] = We1[HID:2 * HID]
        d[f"l{l}_uvW"] = We1[2 * HID:2 * HID + 2]
        d[f"l{l}_We1a"] = We1[0:HID]
        d[f"l{l}_be1"] = np.asarray(p["be1"], np.float32).reshape(HID, 1)
        d[f"l{l}_We2"] = np.asarray(p["We2"], np.float32)
        d[f"l{l}_be2"] = np.asarray(p["be2"], np.float32).reshape(HID, 1)
        WxM = np.zeros((HID, 97), dtype=np.float32)
        if l < L - 1:
            WxM[:, 0:HID] = np.asarray(p["Wx1"], np.float32)
        WxM[:, 96] = np.asarray(p["Wm"], np.float32)[:, 0]
        d[f"l{l}_WxM"] = WxM
        d[f"l{l}_bx1"] = (np.asarray(p["bx1"], np.float32).reshape(HID, 1)
                          if l < L - 1 else np.zeros((HID, 1), np.float32))
        d[f"l{l}_bm97"] = np.full((97, 1), float(np.asarray(p["bm"])[0]),
                                  dtype=np.float32)
        d[f"l{l}_Wx2"] = (np.asarray(p["Wx2"], np.float32)
                          if l < L - 1 else np.zeros((HID, 1), np.float32))
        Wh1 = np.asarray(p["Wh1"], dtype=np.float32)          # [144, 72]
        d[f"l{l}_Wh1a"] = Wh1[0:HID]
        d[f"l{l}_Wh1b"] = Wh1[HID:2 * HID]
        d[f"l{l}_bh1"] = np.asarray(p["bh1"], np.float32).reshape(HID, 1)
        d[f"l{l}_Wh2"] = np.asarray(p["Wh2"], np.float32)
        d[f"l{l}_bh2"] = np.asarray(p["bh2"], np.float32).reshape(HID, 1)

    # pack all consts into the single cpack tensor (+ bf16 sidecar)
    import ml_dtypes
    specs = const_layout(n_ev, n_layers)
    W = sum(w for _, _, w in specs.values())
    cp = np.zeros((P, W), dtype=np.float32)
    for off, (name, (_, p_, w_)) in zip(_offsets(specs), specs.items()):
        arr = d[name]
        assert arr.shape == (p_, w_), (name, arr.shape, (p_, w_))
        cp[0:p_, off:off + w_] = arr
    hspecs = const_layout_h(n_layers)
    Wh = sum(w for _, w in hspecs.values())
    cph = np.zeros((P, Wh), dtype=ml_dtypes.bfloat16)
    hoff = 0
    for name, (p_, w_) in hspecs.items():
        arr = d[name]
        assert arr.shape == (p_, w_), (name, arr.shape, (p_, w_))
        cph[0:p_, hoff:hoff + w_] = arr.astype(ml_dtypes.bfloat16)
        hoff += w_
    for name in list(d.keys()):
        if name not in ("momT", "scalT"):
            d.pop(name)
    d["cpack"] = cp
    d["cpackh"] = cph
    return d


_PROGRAM_CACHE = {}


def kernel(mom4, mask, scalars, edge_i, edge_j, params):
    from concourse.bass_utils import run_bass_kernel_spmd

    mom4 = np.asarray(mom4)
    mask = np.asarray(mask)
    scalars = np.asarray(scalars)

    key = ("full", EV, L)
    if key not in _PROGRAM_CACHE:
        _PROGRAM_CACHE[key] = build_program(EV, L)
    nc = _PROGRAM_CACHE[key]

    in_maps = [pack_inputs(mom4, mask, scalars, params, core)
               for core in range(N_CORES)]
    res = run_bass_kernel_spmd(nc, in_maps, list(range(N_CORES)))
    out = np.concatenate([res.results[c]["out"] for c in range(N_CORES)], axis=0)
    return out.astype(np.float32)
